# revision 6
# baseline (speedup 1.0000x reference)
"""MultiInnerProductDecoder on 8 trn2 NeuronCores.

For each edge type t (8 of them) and edge e:
    out[t, e] = sigmoid( sum_d z[src[t,e], d] * z[dst[t,e], d] * weight[t, d] )

Sharding: one edge type per core. z is replicated; each core gathers
2 x 100k z-rows (512B each) from its HBM copy with InstDMAGatherAnt
(SWDGE firmware gather, 4 queues), computes the weighted inner product
on DVE, sigmoid on ACT.

dma_gather takes int16 indices, so node ids are split into four
32768-row chunks. The host sorts each core's edges into 16
(src_chunk, dst_chunk) buckets; within a bucket both endpoints are
addressed as `id & 0x7fff` against compile-time chunk bases. Buckets
are padded (with node-0 dummy edges) to the max bucket size across the
8 cores so a single SPMD program serves all cores. Bucket data is
processed in segments of up to SEG_E edges; gather position i lands at
out[i % 128, i // 128, :]. The host inverse-permutes the result.
"""

import math

import numpy as np

import concourse.mybir as mybir
from concourse import bacc
from concourse.bass_utils import run_bass_kernel_spmd
from concourse.tile import TileContext

N_NODES = 100000
NUM_ET = 8
D = 128
N_EDGES = 100000

CHUNK = 32768               # rows per index chunk (int16 reach)
N_CHUNKS = 4
SEG_E = 4096                # max edges per gather segment
SEG_C = SEG_E // 128        # free-dim slots per partition at full segment
NQ = 4                      # SWDGE queues

F32 = mybir.dt.float32
I16 = mybir.dt.int16


def build_program(seg_specs):
    """seg_specs: tuple of (src_chunk, dst_chunk, num_idxs) per segment,
    num_idxs a multiple of 128, <= SEG_E. Same for all cores."""
    n_seg = len(seg_specs)
    nc = bacc.Bacc(num_swdge_queues=NQ)

    z = nc.declare_dram_parameter("z", [N_NODES, D], F32, isOutput=False)
    w_tiled = nc.declare_dram_parameter("w_tiled", [128, SEG_C * D], F32, isOutput=False)
    idx_src = nc.declare_dram_parameter("idx_src", [n_seg * 128, SEG_E // 16], I16, isOutput=False)
    idx_dst = nc.declare_dram_parameter("idx_dst", [n_seg * 128, SEG_E // 16], I16, isOutput=False)
    out = nc.declare_dram_parameter("out", [n_seg * 128, SEG_C], F32, isOutput=True)

    with TileContext(nc) as tc:
        with (
            tc.tile_pool(name="const", bufs=1) as const_pool,
            tc.tile_pool(name="idx", bufs=4) as idx_pool,
            tc.tile_pool(name="work", bufs=3) as work,
            tc.tile_pool(name="small", bufs=4) as small,
        ):
            w_tile = const_pool.tile([128, SEG_C * D], F32)
            nc.sync.dma_start(out=w_tile[:], in_=w_tiled[:])

            for s, (cs, cd, ni) in enumerate(seg_specs):
                c = ni // 128
                rows = slice(s * 128, (s + 1) * 128)

                si = idx_pool.tile([128, ni // 16], I16, tag="si")
                di = idx_pool.tile([128, ni // 16], I16, tag="di")
                nc.sync.dma_start(out=si[:], in_=idx_src[rows, : ni // 16])
                nc.sync.dma_start(out=di[:], in_=idx_dst[rows, : ni // 16])

                zs = work.tile([128, c * D], F32, tag="zs")
                zd = work.tile([128, c * D], F32, tag="zd")
                src_base = cs * CHUNK
                dst_base = cd * CHUNK
                nc.gpsimd.dma_gather(
                    out_ap=zs[:].rearrange("p (c d) -> p c d", d=D),
                    in_ap=z[src_base : min(src_base + CHUNK, N_NODES), :],
                    idxs_ap=si[:],
                    num_idxs=ni,
                    num_idxs_reg=ni,
                    elem_size=D,
                    single_packet=False,
                    queue_num=(2 * s) % NQ,
                )
                nc.gpsimd.dma_gather(
                    out_ap=zd[:].rearrange("p (c d) -> p c d", d=D),
                    in_ap=z[dst_base : min(dst_base + CHUNK, N_NODES), :],
                    idxs_ap=di[:],
                    num_idxs=ni,
                    num_idxs_reg=ni,
                    elem_size=D,
                    single_packet=False,
                    queue_num=(2 * s + 1) % NQ,
                )

                # zs <- zs * w ; zs <- zs * zd ; vals <- sum_d ; sigmoid
                nc.vector.tensor_tensor(
                    out=zs[:], in0=zs[:], in1=w_tile[:, : c * D], op=mybir.AluOpType.mult
                )
                nc.vector.tensor_tensor(
                    out=zs[:], in0=zs[:], in1=zd[:], op=mybir.AluOpType.mult
                )
                vals = small.tile([128, c], F32, tag="vals")
                nc.vector.tensor_reduce(
                    out=vals[:],
                    in_=zs[:].rearrange("p (c d) -> p c d", d=D),
                    axis=mybir.AxisListType.X,
                    op=mybir.AluOpType.add,
                )
                res = small.tile([128, c], F32, tag="res")
                nc.scalar.activation(
                    out=res[:], in_=vals[:], func=mybir.ActivationFunctionType.Sigmoid
                )
                nc.sync.dma_start(out=out[rows, :c], in_=res[:])

    nc.finalize()
    return nc


_PROGRAMS = {}


def _get_program(seg_specs):
    key = tuple(seg_specs)
    if key not in _PROGRAMS:
        _PROGRAMS[key] = build_program(seg_specs)
    return _PROGRAMS[key]


def _wrap16(flat):
    """[n] int16 gather-position order -> [128, n/16] wrapped+replicated."""
    n = flat.shape[0]
    w = flat.reshape(n // 16, 16).T  # [16, n/16]; position i at [i%16, i//16]
    return np.tile(w, (8, 1))


def prepare(z, weight, edge_src, edge_dst):
    """Host-side bucketing. Returns (in_maps, seg_specs, recover)."""
    z = np.ascontiguousarray(np.asarray(z, dtype=np.float32))
    weight = np.asarray(weight, dtype=np.float32)
    src = np.asarray(edge_src, dtype=np.int64)
    dst = np.asarray(edge_dst, dtype=np.int64)
    n_edges = src.shape[1]

    key = (src >> 15) * N_CHUNKS + (dst >> 15)          # [T, E] bucket 0..15
    orders = [np.argsort(key[t], kind="stable") for t in range(NUM_ET)]
    counts = np.stack(
        [np.bincount(key[t], minlength=16) for t in range(NUM_ET)]
    )  # [T, 16]
    gsize = counts.max(axis=0)                           # padded bucket sizes
    gsize = ((gsize + 127) // 128) * 128
    gbase = np.concatenate([[0], np.cumsum(gsize)])      # [17]
    total = int(gbase[-1])

    # segment layout (same for all cores)
    seg_specs = []
    seg_group_off = []                                   # (group, offset) per seg
    for g in range(16):
        sz = int(gsize[g])
        off = 0
        while off < sz:
            ni = min(SEG_E, sz - off)
            seg_specs.append((g // N_CHUNKS, g % N_CHUNKS, ni))
            seg_group_off.append((g, off))
            off += ni
    n_seg = len(seg_specs)

    in_maps = []
    padpos_all = []
    for t in range(NUM_ET):
        order = orders[t]
        cnt = counts[t]
        # padded position for each sorted edge
        within = np.concatenate([np.arange(cnt[g]) for g in range(16)])
        bases = np.repeat(gbase[:16], cnt)
        padpos_sorted = bases + within
        padpos = np.empty(n_edges, dtype=np.int64)
        padpos[order] = padpos_sorted
        padpos_all.append(padpos)

        src_loc = np.zeros(total, dtype=np.int16)
        dst_loc = np.zeros(total, dtype=np.int16)
        src_loc[padpos] = (src[t] & 0x7FFF).astype(np.int16)
        dst_loc[padpos] = (dst[t] & 0x7FFF).astype(np.int16)

        idx_src_np = np.zeros((n_seg * 128, SEG_E // 16), dtype=np.int16)
        idx_dst_np = np.zeros((n_seg * 128, SEG_E // 16), dtype=np.int16)
        for s, ((g, off), (_, _, ni)) in enumerate(zip(seg_group_off, seg_specs)):
            lo = int(gbase[g]) + off
            idx_src_np[s * 128 : (s + 1) * 128, : ni // 16] = _wrap16(
                src_loc[lo : lo + ni]
            )
            idx_dst_np[s * 128 : (s + 1) * 128, : ni // 16] = _wrap16(
                dst_loc[lo : lo + ni]
            )

        in_maps.append(
            {
                "z": z,
                "w_tiled": np.tile(weight[t], (128, SEG_C)).astype(np.float32),
                "idx_src": idx_src_np,
                "idx_dst": idx_dst_np,
            }
        )

    recover = (seg_specs, seg_group_off, gbase, padpos_all, n_edges)
    return in_maps, tuple(seg_specs), recover


def recover_output(results, recover):
    seg_specs, seg_group_off, gbase, padpos_all, n_edges = recover
    total = int(gbase[-1])
    outs = []
    for t in range(NUM_ET):
        out_dram = results[t]["out"]                     # [n_seg*128, SEG_C]
        vals_padded = np.empty(total, dtype=np.float32)
        for s, ((g, off), (_, _, ni)) in enumerate(zip(seg_group_off, seg_specs)):
            seg = out_dram[s * 128 : (s + 1) * 128, : ni // 128]
            lo = int(gbase[g]) + off
            vals_padded[lo : lo + ni] = seg.T.ravel()    # position i at [i%128,i//128]
        outs.append(vals_padded[padpos_all[t]])
    return np.stack(outs).astype(np.float32)


def kernel(z, weight, edge_src, edge_dst):
    in_maps, seg_specs, recover = prepare(z, weight, edge_src, edge_dst)
    nc = _get_program(seg_specs)
    res = run_bass_kernel_spmd(nc, in_maps, core_ids=list(range(NUM_ET)))
    return recover_output(res.results, recover)


# revision 8
# speedup vs baseline: 1.2253x; 1.2253x over previous
"""MultiInnerProductDecoder on 8 trn2 NeuronCores.

For each edge type t (8 of them) and edge e:
    out[t, e] = sigmoid( sum_d z[src[t,e], d] * z[dst[t,e], d] * weight[t, d] )

Sharding: one edge type per core. z is replicated; each core gathers
2 x 100k z-rows (512B each) from its HBM copy with InstDMAGatherAnt
(SWDGE firmware gather, 4 queues), computes the weighted inner product
on DVE, sigmoid on ACT.

dma_gather takes int16 indices, so node ids are split into four
32768-row chunks. The host sorts each core's edges into 16
(src_chunk, dst_chunk) buckets; within a bucket both endpoints are
addressed as `id & 0x7fff` against compile-time chunk bases. Buckets
are padded (with node-0 dummy edges) to the max bucket size across the
8 cores so a single SPMD program serves all cores. Bucket data is
processed in segments of up to SEG_E edges; gather position i lands at
out[i % 128, i // 128, :]. The host inverse-permutes the result.
"""

import math

import numpy as np

import concourse.mybir as mybir
from concourse import bacc
from concourse.bass_utils import run_bass_kernel_spmd
from concourse.tile import TileContext

N_NODES = 100000
NUM_ET = 8
D = 128
N_EDGES = 100000

CHUNK = 32768               # rows per index chunk (int16 reach)
N_CHUNKS = 4
SEG_E = 4096                # max edges per gather segment
SEG_C = SEG_E // 128        # free-dim slots per partition at full segment
NQ = 4                      # SWDGE queues

F32 = mybir.dt.float32
I16 = mybir.dt.int16


def build_program(seg_specs):
    """seg_specs: tuple of (src_chunk, dst_chunk, num_idxs) per segment,
    num_idxs a multiple of 128, <= SEG_E. Same for all cores."""
    n_seg = len(seg_specs)
    nc = bacc.Bacc(num_swdge_queues=NQ)

    z = nc.declare_dram_parameter("z", [N_NODES, D], F32, isOutput=False)
    w_tiled = nc.declare_dram_parameter("w_tiled", [128, SEG_C * D], F32, isOutput=False)
    idx_src = nc.declare_dram_parameter("idx_src", [n_seg * 128, SEG_E // 16], I16, isOutput=False)
    idx_dst = nc.declare_dram_parameter("idx_dst", [n_seg * 128, SEG_E // 16], I16, isOutput=False)
    out = nc.declare_dram_parameter("out", [n_seg * 128, SEG_C], F32, isOutput=True)

    with TileContext(nc) as tc:
        with (
            tc.tile_pool(name="const", bufs=1) as const_pool,
            tc.tile_pool(name="work", bufs=4) as work,
            tc.tile_pool(name="small", bufs=4) as small,
        ):
            w_tile = const_pool.tile([128, SEG_C * D], F32)
            nc.sync.dma_start(out=w_tile[:], in_=w_tiled[:])

            # Preload every segment's wrapped indices once — avoids a
            # per-segment idx-DMA dependency ahead of each gather.
            si_all = const_pool.tile([128, n_seg * (SEG_E // 16)], I16)
            di_all = const_pool.tile([128, n_seg * (SEG_E // 16)], I16)
            nc.sync.dma_start(
                out=si_all[:].rearrange("p (s w) -> p s w", s=n_seg),
                in_=idx_src[:].rearrange("(s p) w -> p s w", p=128),
            )
            nc.sync.dma_start(
                out=di_all[:].rearrange("p (s w) -> p s w", s=n_seg),
                in_=idx_dst[:].rearrange("(s p) w -> p s w", p=128),
            )
            si_v = si_all[:].rearrange("p (s w) -> p s w", s=n_seg)
            di_v = di_all[:].rearrange("p (s w) -> p s w", s=n_seg)

            for s, (cs, cd, ni) in enumerate(seg_specs):
                c = ni // 128
                rows = slice(s * 128, (s + 1) * 128)

                zs = work.tile([128, c * D], F32, tag="zs")
                zd = work.tile([128, c * D], F32, tag="zd")
                src_base = cs * CHUNK
                dst_base = cd * CHUNK
                nc.gpsimd.dma_gather(
                    out_ap=zs[:].rearrange("p (c d) -> p c d", d=D),
                    in_ap=z[src_base : min(src_base + CHUNK, N_NODES), :],
                    idxs_ap=si_v[:, s, : ni // 16],
                    num_idxs=ni,
                    num_idxs_reg=ni,
                    elem_size=D,
                    single_packet=False,
                    queue_num=(2 * s) % NQ,
                )
                nc.gpsimd.dma_gather(
                    out_ap=zd[:].rearrange("p (c d) -> p c d", d=D),
                    in_ap=z[dst_base : min(dst_base + CHUNK, N_NODES), :],
                    idxs_ap=di_v[:, s, : ni // 16],
                    num_idxs=ni,
                    num_idxs_reg=ni,
                    elem_size=D,
                    single_packet=False,
                    queue_num=(2 * s + 1) % NQ,
                )

                # zs <- zs * w ; zs <- zs * zd ; vals <- sum_d ; sigmoid
                nc.vector.tensor_tensor(
                    out=zs[:], in0=zs[:], in1=w_tile[:, : c * D], op=mybir.AluOpType.mult
                )
                nc.vector.tensor_tensor(
                    out=zs[:], in0=zs[:], in1=zd[:], op=mybir.AluOpType.mult
                )
                vals = small.tile([128, c], F32, tag="vals")
                nc.vector.tensor_reduce(
                    out=vals[:],
                    in_=zs[:].rearrange("p (c d) -> p c d", d=D),
                    axis=mybir.AxisListType.X,
                    op=mybir.AluOpType.add,
                )
                res = small.tile([128, c], F32, tag="res")
                nc.scalar.activation(
                    out=res[:], in_=vals[:], func=mybir.ActivationFunctionType.Sigmoid
                )
                nc.sync.dma_start(out=out[rows, :c], in_=res[:])

    # Tile round-robins the 8 DMASW sem lanes over Pool-DMA instructions in
    # scheduled order, and the SWDGE firmware requires each sem lane to stay
    # on one queue. Derive queue_num from the assigned lane so the pairing
    # is consistent and the 4 queues stay evenly loaded.
    for f in nc.m.functions:
        for b in f.blocks:
            for ins in b.instructions:
                if isinstance(ins, mybir.InstDMAGatherAnt) and ins.sync_info:
                    for u in ins.sync_info.on_update or []:
                        name = getattr(u, "ant_name", "") or ""
                        if name.startswith("DMASW"):
                            ins.queue_num = int(name[5:].split("_")[0]) % NQ
                            break

    nc.finalize()
    return nc


_PROGRAMS = {}


def _get_program(seg_specs):
    key = tuple(seg_specs)
    if key not in _PROGRAMS:
        _PROGRAMS[key] = build_program(seg_specs)
    return _PROGRAMS[key]


def _wrap16(flat):
    """[n] int16 gather-position order -> [128, n/16] wrapped+replicated."""
    n = flat.shape[0]
    w = flat.reshape(n // 16, 16).T  # [16, n/16]; position i at [i%16, i//16]
    return np.tile(w, (8, 1))


def prepare(z, weight, edge_src, edge_dst):
    """Host-side bucketing. Returns (in_maps, seg_specs, recover)."""
    z = np.ascontiguousarray(np.asarray(z, dtype=np.float32))
    weight = np.asarray(weight, dtype=np.float32)
    src = np.asarray(edge_src, dtype=np.int64)
    dst = np.asarray(edge_dst, dtype=np.int64)
    n_edges = src.shape[1]

    key = (src >> 15) * N_CHUNKS + (dst >> 15)          # [T, E] bucket 0..15
    orders = [np.argsort(key[t], kind="stable") for t in range(NUM_ET)]
    counts = np.stack(
        [np.bincount(key[t], minlength=16) for t in range(NUM_ET)]
    )  # [T, 16]
    gsize = counts.max(axis=0)                           # padded bucket sizes
    gsize = ((gsize + 127) // 128) * 128
    gbase = np.concatenate([[0], np.cumsum(gsize)])      # [17]
    total = int(gbase[-1])

    # segment layout (same for all cores)
    seg_specs = []
    seg_group_off = []                                   # (group, offset) per seg
    for g in range(16):
        sz = int(gsize[g])
        off = 0
        while off < sz:
            ni = min(SEG_E, sz - off)
            seg_specs.append((g // N_CHUNKS, g % N_CHUNKS, ni))
            seg_group_off.append((g, off))
            off += ni
    n_seg = len(seg_specs)

    in_maps = []
    padpos_all = []
    for t in range(NUM_ET):
        order = orders[t]
        cnt = counts[t]
        # padded position for each sorted edge
        within = np.concatenate([np.arange(cnt[g]) for g in range(16)])
        bases = np.repeat(gbase[:16], cnt)
        padpos_sorted = bases + within
        padpos = np.empty(n_edges, dtype=np.int64)
        padpos[order] = padpos_sorted
        padpos_all.append(padpos)

        src_loc = np.zeros(total, dtype=np.int16)
        dst_loc = np.zeros(total, dtype=np.int16)
        src_loc[padpos] = (src[t] & 0x7FFF).astype(np.int16)
        dst_loc[padpos] = (dst[t] & 0x7FFF).astype(np.int16)

        idx_src_np = np.zeros((n_seg * 128, SEG_E // 16), dtype=np.int16)
        idx_dst_np = np.zeros((n_seg * 128, SEG_E // 16), dtype=np.int16)
        for s, ((g, off), (_, _, ni)) in enumerate(zip(seg_group_off, seg_specs)):
            lo = int(gbase[g]) + off
            idx_src_np[s * 128 : (s + 1) * 128, : ni // 16] = _wrap16(
                src_loc[lo : lo + ni]
            )
            idx_dst_np[s * 128 : (s + 1) * 128, : ni // 16] = _wrap16(
                dst_loc[lo : lo + ni]
            )

        in_maps.append(
            {
                "z": z,
                "w_tiled": np.tile(weight[t], (128, SEG_C)).astype(np.float32),
                "idx_src": idx_src_np,
                "idx_dst": idx_dst_np,
            }
        )

    recover = (seg_specs, seg_group_off, gbase, padpos_all, n_edges)
    return in_maps, tuple(seg_specs), recover


def recover_output(results, recover):
    seg_specs, seg_group_off, gbase, padpos_all, n_edges = recover
    total = int(gbase[-1])
    outs = []
    for t in range(NUM_ET):
        out_dram = results[t]["out"]                     # [n_seg*128, SEG_C]
        vals_padded = np.empty(total, dtype=np.float32)
        for s, ((g, off), (_, _, ni)) in enumerate(zip(seg_group_off, seg_specs)):
            seg = out_dram[s * 128 : (s + 1) * 128, : ni // 128]
            lo = int(gbase[g]) + off
            vals_padded[lo : lo + ni] = seg.T.ravel()    # position i at [i%128,i//128]
        outs.append(vals_padded[padpos_all[t]])
    return np.stack(outs).astype(np.float32)


def kernel(z, weight, edge_src, edge_dst):
    in_maps, seg_specs, recover = prepare(z, weight, edge_src, edge_dst)
    nc = _get_program(seg_specs)
    res = run_bass_kernel_spmd(nc, in_maps, core_ids=list(range(NUM_ET)))
    return recover_output(res.results, recover)


# revision 11
# speedup vs baseline: 1.5743x; 1.2849x over previous
"""MultiInnerProductDecoder on 8 trn2 NeuronCores.

For each edge type t (8 of them) and edge e:
    out[t, e] = sigmoid( sum_d z[src[t,e], d] * z[dst[t,e], d] * weight[t, d] )

Sharding: one edge type per core. z is replicated; each core gathers
2 x 100k z-rows (512B each) from its HBM copy with InstDMAGatherAnt
(SWDGE firmware gather, 4 queues), computes the weighted inner product
on DVE, sigmoid on ACT.

dma_gather takes int16 indices, so node ids are split into four
32768-row chunks. The host sorts each core's edges into 16
(src_chunk, dst_chunk) buckets; within a bucket both endpoints are
addressed as `id & 0x7fff` against compile-time chunk bases. Buckets
are padded (with node-0 dummy edges) to the max bucket size across the
8 cores so a single SPMD program serves all cores. Bucket data is
processed in segments of up to SEG_E edges; gather position i lands at
out[i % 128, i // 128, :]. The host inverse-permutes the result.
"""

import math

import numpy as np

import concourse.mybir as mybir
from concourse import bacc
from concourse.bass_utils import run_bass_kernel_spmd
from concourse.tile import TileContext

N_NODES = 100000
NUM_ET = 8
D = 128
N_EDGES = 100000

CHUNK = 32768               # rows per index chunk (int16 reach)
N_CHUNKS = 4
SEG_E = int(__import__("os").environ.get("KM_SEG_E", 4096))  # edges per segment
SEG_C = SEG_E // 128        # free-dim slots per partition at full segment
NQ = 4                      # SWDGE queues
SCRATCH = int(__import__("os").environ.get("KM_SCRATCH", 16384))
WORK_BUFS = int(__import__("os").environ.get("KM_WORK_BUFS", 4))

F32 = mybir.dt.float32
I16 = mybir.dt.int16


def build_program(seg_specs):
    """seg_specs: tuple of (src_chunk, dst_chunk, num_idxs) per segment,
    num_idxs a multiple of 128, <= SEG_E. Same for all cores."""
    n_seg = len(seg_specs)
    nc = bacc.Bacc(num_swdge_queues=NQ, dynamic_dma_scratch_size=SCRATCH)

    z = nc.declare_dram_parameter("z", [N_NODES, D], F32, isOutput=False)
    w_tiled = nc.declare_dram_parameter("w_tiled", [128, SEG_C * D], F32, isOutput=False)
    idx_src = nc.declare_dram_parameter("idx_src", [n_seg * 128, SEG_E // 16], I16, isOutput=False)
    idx_dst = nc.declare_dram_parameter("idx_dst", [n_seg * 128, SEG_E // 16], I16, isOutput=False)
    out = nc.declare_dram_parameter("out", [n_seg * 128, SEG_C], F32, isOutput=True)

    with TileContext(nc) as tc:
        with (
            tc.tile_pool(name="const", bufs=1) as const_pool,
            tc.tile_pool(name="work", bufs=WORK_BUFS) as work,
            tc.tile_pool(name="small", bufs=4) as small,
        ):
            w_tile = const_pool.tile([128, SEG_C * D], F32)
            nc.sync.dma_start(out=w_tile[:], in_=w_tiled[:])

            # Preload every segment's wrapped indices once — avoids a
            # per-segment idx-DMA dependency ahead of each gather.
            si_all = const_pool.tile([128, n_seg * (SEG_E // 16)], I16)
            di_all = const_pool.tile([128, n_seg * (SEG_E // 16)], I16)
            nc.sync.dma_start(
                out=si_all[:].rearrange("p (s w) -> p s w", s=n_seg),
                in_=idx_src[:].rearrange("(s p) w -> p s w", p=128),
            )
            nc.sync.dma_start(
                out=di_all[:].rearrange("p (s w) -> p s w", s=n_seg),
                in_=idx_dst[:].rearrange("(s p) w -> p s w", p=128),
            )
            si_v = si_all[:].rearrange("p (s w) -> p s w", s=n_seg)
            di_v = di_all[:].rearrange("p (s w) -> p s w", s=n_seg)

            for s, (cs, cd, ni) in enumerate(seg_specs):
                c = ni // 128
                rows = slice(s * 128, (s + 1) * 128)

                zs = work.tile([128, c * D], F32, tag="zs")
                zd = work.tile([128, c * D], F32, tag="zd")
                src_base = cs * CHUNK
                dst_base = cd * CHUNK
                nc.gpsimd.dma_gather(
                    out_ap=zs[:].rearrange("p (c d) -> p c d", d=D),
                    in_ap=z[src_base : min(src_base + CHUNK, N_NODES), :],
                    idxs_ap=si_v[:, s, : ni // 16],
                    num_idxs=ni,
                    num_idxs_reg=ni,
                    elem_size=D,
                    single_packet=False,
                    queue_num=(2 * s) % NQ,
                )
                nc.gpsimd.dma_gather(
                    out_ap=zd[:].rearrange("p (c d) -> p c d", d=D),
                    in_ap=z[dst_base : min(dst_base + CHUNK, N_NODES), :],
                    idxs_ap=di_v[:, s, : ni // 16],
                    num_idxs=ni,
                    num_idxs_reg=ni,
                    elem_size=D,
                    single_packet=False,
                    queue_num=(2 * s + 1) % NQ,
                )

                # zs <- zs * w ; zs <- zs * zd ; vals <- sum_d ; sigmoid
                nc.vector.tensor_tensor(
                    out=zs[:], in0=zs[:], in1=w_tile[:, : c * D], op=mybir.AluOpType.mult
                )
                nc.vector.tensor_tensor(
                    out=zs[:], in0=zs[:], in1=zd[:], op=mybir.AluOpType.mult
                )
                vals = small.tile([128, c], F32, tag="vals")
                nc.vector.tensor_reduce(
                    out=vals[:],
                    in_=zs[:].rearrange("p (c d) -> p c d", d=D),
                    axis=mybir.AxisListType.X,
                    op=mybir.AluOpType.add,
                )
                res = small.tile([128, c], F32, tag="res")
                nc.scalar.activation(
                    out=res[:], in_=vals[:], func=mybir.ActivationFunctionType.Sigmoid
                )
                nc.sync.dma_start(out=out[rows, :c], in_=res[:])

    # Tile round-robins the 8 DMASW sem lanes over Pool-DMA instructions in
    # scheduled order, and the SWDGE firmware requires each sem lane to stay
    # on one queue. Derive queue_num from the assigned lane so the pairing
    # is consistent and the 4 queues stay evenly loaded.
    for f in nc.m.functions:
        for b in f.blocks:
            for ins in b.instructions:
                if isinstance(ins, mybir.InstDMAGatherAnt) and ins.sync_info:
                    for u in ins.sync_info.on_update or []:
                        name = getattr(u, "ant_name", "") or ""
                        if name.startswith("DMASW"):
                            ins.queue_num = int(name[5:].split("_")[0]) % NQ
                            break

    nc.finalize()
    return nc


_PROGRAMS = {}


def _get_program(seg_specs):
    key = tuple(seg_specs)
    if key not in _PROGRAMS:
        _PROGRAMS[key] = build_program(seg_specs)
    return _PROGRAMS[key]


def _wrap16(flat):
    """[n] int16 gather-position order -> [128, n/16] wrapped+replicated."""
    n = flat.shape[0]
    w = flat.reshape(n // 16, 16).T  # [16, n/16]; position i at [i%16, i//16]
    return np.tile(w, (8, 1))


def prepare(z, weight, edge_src, edge_dst):
    """Host-side bucketing. Returns (in_maps, seg_specs, recover)."""
    z = np.ascontiguousarray(np.asarray(z, dtype=np.float32))
    weight = np.asarray(weight, dtype=np.float32)
    src = np.asarray(edge_src, dtype=np.int64)
    dst = np.asarray(edge_dst, dtype=np.int64)
    n_edges = src.shape[1]

    key = (src >> 15) * N_CHUNKS + (dst >> 15)          # [T, E] bucket 0..15
    orders = [np.argsort(key[t], kind="stable") for t in range(NUM_ET)]
    counts = np.stack(
        [np.bincount(key[t], minlength=16) for t in range(NUM_ET)]
    )  # [T, 16]
    gsize = counts.max(axis=0)                           # padded bucket sizes
    gsize = ((gsize + 127) // 128) * 128
    gbase = np.concatenate([[0], np.cumsum(gsize)])      # [17]
    total = int(gbase[-1])

    # segment layout (same for all cores)
    seg_specs = []
    seg_group_off = []                                   # (group, offset) per seg
    for g in range(16):
        sz = int(gsize[g])
        off = 0
        while off < sz:
            ni = min(SEG_E, sz - off)
            seg_specs.append((g // N_CHUNKS, g % N_CHUNKS, ni))
            seg_group_off.append((g, off))
            off += ni
    n_seg = len(seg_specs)

    in_maps = []
    padpos_all = []
    for t in range(NUM_ET):
        order = orders[t]
        cnt = counts[t]
        # padded position for each sorted edge
        within = np.concatenate([np.arange(cnt[g]) for g in range(16)])
        bases = np.repeat(gbase[:16], cnt)
        padpos_sorted = bases + within
        padpos = np.empty(n_edges, dtype=np.int64)
        padpos[order] = padpos_sorted
        padpos_all.append(padpos)

        src_loc = np.zeros(total, dtype=np.int16)
        dst_loc = np.zeros(total, dtype=np.int16)
        src_loc[padpos] = (src[t] & 0x7FFF).astype(np.int16)
        dst_loc[padpos] = (dst[t] & 0x7FFF).astype(np.int16)

        idx_src_np = np.zeros((n_seg * 128, SEG_E // 16), dtype=np.int16)
        idx_dst_np = np.zeros((n_seg * 128, SEG_E // 16), dtype=np.int16)
        for s, ((g, off), (_, _, ni)) in enumerate(zip(seg_group_off, seg_specs)):
            lo = int(gbase[g]) + off
            idx_src_np[s * 128 : (s + 1) * 128, : ni // 16] = _wrap16(
                src_loc[lo : lo + ni]
            )
            idx_dst_np[s * 128 : (s + 1) * 128, : ni // 16] = _wrap16(
                dst_loc[lo : lo + ni]
            )

        in_maps.append(
            {
                "z": z,
                "w_tiled": np.tile(weight[t], (128, SEG_C)).astype(np.float32),
                "idx_src": idx_src_np,
                "idx_dst": idx_dst_np,
            }
        )

    recover = (seg_specs, seg_group_off, gbase, padpos_all, n_edges)
    return in_maps, tuple(seg_specs), recover


def recover_output(results, recover):
    seg_specs, seg_group_off, gbase, padpos_all, n_edges = recover
    total = int(gbase[-1])
    outs = []
    for t in range(NUM_ET):
        out_dram = results[t]["out"]                     # [n_seg*128, SEG_C]
        vals_padded = np.empty(total, dtype=np.float32)
        for s, ((g, off), (_, _, ni)) in enumerate(zip(seg_group_off, seg_specs)):
            seg = out_dram[s * 128 : (s + 1) * 128, : ni // 128]
            lo = int(gbase[g]) + off
            vals_padded[lo : lo + ni] = seg.T.ravel()    # position i at [i%128,i//128]
        outs.append(vals_padded[padpos_all[t]])
    return np.stack(outs).astype(np.float32)


def kernel(z, weight, edge_src, edge_dst):
    in_maps, seg_specs, recover = prepare(z, weight, edge_src, edge_dst)
    nc = _get_program(seg_specs)
    res = run_bass_kernel_spmd(nc, in_maps, core_ids=list(range(NUM_ET)))
    return recover_output(res.results, recover)
